# revision 6
# baseline (speedup 1.0000x reference)
"""Trainium2 Bass kernel for the kNN pairwise-ranking loss.

Math: with y = (knn_tgts == tgts), the masked pairwise BCE-with-logits loss
over differing-label pairs (j > i) collapses to

    loss = sum_b sum_{n in neg_b} sum_{p in pos_b} softplus(s_n - s_p) / cnt
    cnt  = sum_b |pos_b| * |neg_b|

because for a (pos, neg) pair the per-pair term is softplus(s_neg - s_pos)
regardless of orientation, and b2 cancels in score differences.

Host side: per batch row, permute keys so positives come first, then
negatives, then masked-out entries.  Additive pad vectors (+PAD on
non-positives, -PAD on non-negatives) push padded scores far out so their
softplus contribution underflows to exactly ln(1) = 0; the device then just
sums a dense [pos-chunks x neg-width] softplus block with no masking.

Device (SPMD over 8 cores, 4 batch rows each):
  phase A (per row): h = relu(W1 @ keys^T + b1) via PE (bf16 in, f32 psum),
                     s_row [1,K] via two small PE matmuls off the bf16 h.
  phase B (per row): softplus(s_n - s_p) = ln(1 + e^{s_n} * e^{-s_p}).
                     Row exps e^{s_neg+negpad} [1,nw] and e^{-(s_pos+pospad)}
                     [1,Jmax] on ACT (bf16 out), outer products via K=1 bf16
                     matmuls into PSUM, then one Ln(x+1) pass per pos-chunk
                     reading PSUM with accum_out giving per-partition sums.
Exp and Ln are forced into the single combined ACT table set (see
_patch_act_tables) so only one ACT_TABLE_LOAD is issued.
Host gathers [128, rows*chunks] partial sums, reduces, divides by cnt.
"""

import numpy as np

B, K, D, H = 32, 1024, 1024, 100
N_CORES = 8
BPC = B // N_CORES  # batch rows per core
PAD = 60.0
USE_BF16 = True

_cache = {}
_act_patched = False


def _patch_act_tables():
    """Make Exp/Ln resolve to the single combined ACT table set.

    bass_rust's act-table-load inserter picks, per activation, some set
    containing the needed function; Exp and Ln naturally resolve to two
    different sets, causing a ~1.3us ACT_TABLE_LOAD on every exp<->ln
    transition.  natural_log_exp_and_others contains both, so restricting
    the registry to it for exp/ln yields exactly one load per kernel.
    """
    global _act_patched
    if _act_patched:
        return
    import concourse.bacc as bacc
    import concourse.hw_specs as hw_specs
    import concourse.mybir as mybir

    orig = hw_specs.get_activation_tables
    combined = "natural_log_exp_and_others"

    def patched(arch):
        tabs = orig(arch)
        out = {}
        for name, funcs in tabs.items():
            f = set(funcs)
            if name != combined and combined in tabs:
                f.discard(mybir.ActivationFunctionType.Exp)
                f.discard(mybir.ActivationFunctionType.Ln)
            out[name] = f
        return out

    hw_specs.get_activation_tables = patched
    bacc.get_activation_tables = patched
    _act_patched = True


def _build_program(Jmax, nst, use_bf16):
    import concourse.bacc as bacc
    import concourse.mybir as mybir
    import concourse.tile as tile

    _patch_act_tables()

    f32 = mybir.dt.float32
    bf16 = mybir.dt.bfloat16
    kdt = bf16 if use_bf16 else f32
    npch = (Jmax + 127) // 128  # positive-side partition chunks
    nw = K - nst  # negative-side free width

    nc = bacc.Bacc(
        "TRN2",
        target_bir_lowering=False,
        debug=False,
        enable_asserts=False,
        num_devices=N_CORES,
    )

    keys_d = nc.dram_tensor("keys_t", [BPC, D, K], kdt, kind="ExternalInput").ap()
    w1t_d = nc.dram_tensor("w1t", [D, H], kdt, kind="ExternalInput").ap()
    w2_d = nc.dram_tensor("w2c", [H, 1], kdt, kind="ExternalInput").ap()
    b1_d = nc.dram_tensor("b1c", [H, 1], f32, kind="ExternalInput").ap()
    ppr_d = nc.dram_tensor("ppr", [BPC, Jmax], f32, kind="ExternalInput").ap()
    negrow_d = nc.dram_tensor("negrow", [BPC, nw], f32, kind="ExternalInput").ap()
    out_d = nc.dram_tensor(
        "acc_out", [128, BPC * npch], f32, kind="ExternalOutput"
    ).ap()

    # free-dim slice boundaries for <=512-wide matmul moving operands
    nsl = [(s, min(s + 512, nw)) for s in range(0, nw, 512)]

    with tile.TileContext(nc) as tc:
        with (
            tc.tile_pool(name="const", bufs=1) as cpool,
            tc.tile_pool(name="keys", bufs=8) as kpool,
            tc.tile_pool(name="h", bufs=3) as hpool,
            tc.tile_pool(name="svec", bufs=2) as spool,
            tc.tile_pool(name="tsp", bufs=3) as tpool,
            tc.tile_pool(name="hp", bufs=1, space="PSUM") as hp_pool,
            tc.tile_pool(name="sp", bufs=1, space="PSUM") as sp_pool,
            tc.tile_pool(name="tp", bufs=2, space="PSUM") as tp_pool,
        ):
            # ---- constants ----
            w1t_sb = cpool.tile([128, 8 * H], kdt, tag="w1t")
            for dc in range(8):
                nc.sync.dma_start(
                    w1t_sb[:, dc * H : (dc + 1) * H],
                    w1t_d[dc * 128 : (dc + 1) * 128, :],
                )
            w2_sb = cpool.tile([H, 1], kdt, tag="w2")
            nc.sync.dma_start(w2_sb[:], w2_d[:])
            b1_sb = cpool.tile([H, 1], f32, tag="b1")
            nc.sync.dma_start(b1_sb[:], b1_d[:])
            acc_sb = cpool.tile([128, BPC * npch], f32, tag="acc")

            for b in range(BPC):
                # ---- phase A: MLP scores ----
                hp = hp_pool.tile([H, 1024], f32, tag="hp")
                for dc in range(8):
                    kt = kpool.tile([128, K], kdt, tag="keys")
                    nc.sync.dma_start(kt[:], keys_d[b, dc * 128 : (dc + 1) * 128, :])
                    w_sl = w1t_sb[:, dc * H : (dc + 1) * H]
                    nc.tensor.matmul(
                        hp[:, 0:512], lhsT=w_sl, rhs=kt[:, 0:512],
                        start=(dc == 0), stop=(dc == 7),
                    )
                    nc.tensor.matmul(
                        hp[:, 512:1024], lhsT=w_sl, rhs=kt[:, 512:1024],
                        start=(dc == 0), stop=(dc == 7),
                    )
                # relu(h + b1): PSUM -> SBUF on DVE (cast to kdt for matmul-2)
                h0 = hpool.tile([H, 512], kdt, tag="h")
                h1 = hpool.tile([H, 512], kdt, tag="h")
                nc.vector.tensor_scalar(
                    h0[:], hp[:, 0:512], b1_sb[:], 0.0,
                    op0=mybir.AluOpType.add, op1=mybir.AluOpType.max,
                )
                nc.vector.tensor_scalar(
                    h1[:], hp[:, 512:1024], b1_sb[:], 0.0,
                    op0=mybir.AluOpType.add, op1=mybir.AluOpType.max,
                )
                # s_row over the full row
                sr_ps = sp_pool.tile([1, 1024], f32, tag="sr")
                nc.tensor.matmul(
                    sr_ps[0:1, 0:512], lhsT=w2_sb[:], rhs=h0[:],
                    start=True, stop=True,
                )
                nc.tensor.matmul(
                    sr_ps[0:1, 512:1024], lhsT=w2_sb[:], rhs=h1[:],
                    start=True, stop=True,
                )
                # padded score rows
                ppr_sb = spool.tile([1, Jmax], f32, tag="ppr")
                nc.sync.dma_start(ppr_sb[:], ppr_d[b : b + 1, :])
                ngr_sb = spool.tile([1, nw], f32, tag="ngr")
                nc.sync.dma_start(ngr_sb[:], negrow_d[b : b + 1, :])
                spp_sb = spool.tile([1, Jmax], f32, tag="spp")
                nc.vector.tensor_add(spp_sb[:], sr_ps[0:1, 0:Jmax], ppr_sb[:])
                snr_sb = spool.tile([1, nw], f32, tag="snr")
                nc.vector.tensor_add(snr_sb[:], sr_ps[0:1, nst:K], ngr_sb[:])
                # e^{-(s_pos+pad)} [1,Jmax], e^{s_neg+pad} [1,nw], bf16 out
                ecol_sb = spool.tile([1, Jmax], bf16, tag="ecol")
                nc.scalar.activation(
                    ecol_sb[:], spp_sb[:],
                    mybir.ActivationFunctionType.Exp, scale=-1.0,
                )
                erow_sb = spool.tile([1, nw], bf16, tag="erow")
                nc.scalar.activation(
                    erow_sb[:], snr_sb[:],
                    mybir.ActivationFunctionType.Exp, scale=1.0,
                )
                # ---- phase B: outer products + ln(1+x) with accumulate ----
                for c in range(npch):
                    tp_ps = tp_pool.tile([128, nw], f32, tag="tp")
                    lw = ecol_sb[0:1, c * 128 : (c + 1) * 128]
                    for s0, s1 in nsl:
                        nc.tensor.matmul(
                            tp_ps[:, s0:s1], lhsT=lw, rhs=erow_sb[0:1, s0:s1],
                            start=True, stop=True,
                        )
                    tt = tpool.tile([128, nw], f32, tag="tsp")
                    nc.scalar.activation(
                        tt[:], tp_ps[:],
                        mybir.ActivationFunctionType.Ln,
                        bias=1.0, scale=1.0,
                        accum_out=acc_sb[:, b * npch + c : b * npch + c + 1],
                    )

            nc.sync.dma_start(out_d[:], acc_sb[:])

    nc.compile()
    return nc


def kernel(keys, tgts, knn_tgts, mask, W1, b1, W2, b2, _profile=False):
    import ml_dtypes

    from concourse.bass_utils import run_bass_kernel_spmd

    keys = np.asarray(keys, dtype=np.float32)
    tgts = np.asarray(tgts)
    knn_tgts = np.asarray(knn_tgts)
    mask = np.asarray(mask).astype(bool)
    W1 = np.asarray(W1, dtype=np.float32)
    b1 = np.asarray(b1, dtype=np.float32)
    W2 = np.asarray(W2, dtype=np.float32)

    # ---- host-side label/permutation prep ----
    y = knn_tgts == tgts[:, None]
    pos = y & mask
    neg = (~y) & mask
    P = pos.sum(axis=1)
    N_ = neg.sum(axis=1)
    cnt = float((P.astype(np.int64) * N_.astype(np.int64)).sum())

    # stable order: positives, negatives, masked-out
    rank = np.where(pos, 0, np.where(neg, 1, 2)).astype(np.int8)
    order = np.argsort(rank, axis=1, kind="stable")  # [B, K]

    Pmax = int(P.max())
    Pmin = int(P.min())
    assert Pmax <= 512, f"positive count {Pmax} > 512 unsupported"
    Jmax = min(512, ((Pmax + 127) // 128) * 128)
    npch = (Jmax + 127) // 128
    nst = min(Pmin, 512)  # negative free region start (s_row slice origin)
    nw = K - nst

    # permuted, transposed keys: [B, D, K]
    keys_perm = np.take_along_axis(keys, order[:, :, None], axis=1)  # [B, K, D]
    keys_t = np.ascontiguousarray(keys_perm.transpose(0, 2, 1))
    kdt = ml_dtypes.bfloat16 if USE_BF16 else np.float32
    keys_t = keys_t.astype(kdt)

    # pads in permuted coordinates
    kidx = np.arange(K)[None, :]
    pospad = np.where(kidx < P[:, None], 0.0, PAD).astype(np.float32)  # [B, K]
    negpad = np.where(
        (kidx >= P[:, None]) & (kidx < (P + N_)[:, None]), 0.0, -PAD
    ).astype(np.float32)
    ppr = np.ascontiguousarray(pospad[:, :Jmax])  # [B, Jmax]
    negrow = np.ascontiguousarray(negpad[:, nst:])  # [B, nw]

    w1t = np.ascontiguousarray(W1.T).astype(kdt)  # [D, H]
    w2c = np.ascontiguousarray(W2.reshape(1, H).T).astype(kdt)  # [H, 1]
    b1c = np.ascontiguousarray(b1.reshape(H, 1))

    key = (Jmax, nst, USE_BF16)
    if key not in _cache:
        _cache[key] = _build_program(Jmax, nst, USE_BF16)
    nc = _cache[key]

    in_maps = []
    for c in range(N_CORES):
        sl = slice(c * BPC, (c + 1) * BPC)
        in_maps.append(
            {
                "keys_t": keys_t[sl],
                "w1t": w1t,
                "w2c": w2c,
                "b1c": b1c,
                "ppr": ppr[sl],
                "negrow": negrow[sl],
            }
        )

    res = run_bass_kernel_spmd(
        nc, in_maps, list(range(N_CORES)), trace=bool(_profile)
    )
    total = 0.0
    for r in res.results:
        total += float(r["acc_out"].astype(np.float64).sum())
    if _profile:
        print(f"HW exec time: {res.exec_time_ns} ns")
        globals()["_last_results"] = res
    loss = np.float64(total) / np.float64(cnt)
    return np.array(loss, dtype=np.float32)


# revision 7
# speedup vs baseline: 1.0055x; 1.0055x over previous
"""Trainium2 Bass kernel for the kNN pairwise-ranking loss.

Math: with y = (knn_tgts == tgts), the masked pairwise BCE-with-logits loss
over differing-label pairs (j > i) collapses to

    loss = sum_b sum_{n in neg_b} sum_{p in pos_b} softplus(s_n - s_p) / cnt
    cnt  = sum_b |pos_b| * |neg_b|

because for a (pos, neg) pair the per-pair term is softplus(s_neg - s_pos)
regardless of orientation, and b2 cancels in score differences.

Host side: per batch row, permute keys so positives come first, then
negatives, then masked-out entries.  Additive pad vectors (+PAD on
non-positives, -PAD on non-negatives) push padded scores far out so their
softplus contribution underflows to exactly ln(1) = 0; the device then just
sums a dense [pos-chunks x neg-width] softplus block with no masking.

Device (SPMD over 8 cores, 4 batch rows each):
  phase A (per row): h = relu(W1 @ keys^T + b1) via PE (bf16 in, f32 psum),
                     s_row [1,K] via two small PE matmuls off the bf16 h.
  phase B (per row): softplus(s_n - s_p) = ln(1 + e^{s_n} * e^{-s_p}).
                     One fused ACT exp produces both e^{-(s_pos+pad)} [1,Jmax]
                     and e^{s_neg+pad} [1,nw]; GPSIMD partition-broadcasts the
                     row factor, a DRAM round-trip reshapes the column factor
                     to [128,npch]; DVE per-partition multiplies form the
                     outer products; one Ln(x+1) pass per row with accum_out
                     yields the per-partition sums.  Exp and Ln are forced
                     into one ACT table set (see _patch_act_tables) so only a
                     single ACT_TABLE_LOAD is issued.
Host gathers [128, BPC] partial sums, reduces, divides by cnt.
"""

import numpy as np

B, K, D, H = 32, 1024, 1024, 100
N_CORES = 8
BPC = B // N_CORES  # batch rows per core
PAD = 60.0
USE_BF16 = True

_cache = {}
_act_patched = False


def _patch_act_tables():
    """Make Exp/Ln resolve to the single combined ACT table set."""
    global _act_patched
    if _act_patched:
        return
    import concourse.bacc as bacc
    import concourse.hw_specs as hw_specs
    import concourse.mybir as mybir

    orig = hw_specs.get_activation_tables
    combined = "natural_log_exp_and_others"

    def patched(arch):
        tabs = orig(arch)
        out = {}
        for name, funcs in tabs.items():
            f = set(funcs)
            if name != combined and combined in tabs:
                f.discard(mybir.ActivationFunctionType.Exp)
                f.discard(mybir.ActivationFunctionType.Ln)
            out[name] = f
        return out

    hw_specs.get_activation_tables = patched
    bacc.get_activation_tables = patched
    _act_patched = True


def _build_program(Jmax, nst, use_bf16):
    import concourse.bacc as bacc
    import concourse.mybir as mybir
    import concourse.tile as tile

    _patch_act_tables()

    f32 = mybir.dt.float32
    kdt = mybir.dt.bfloat16 if use_bf16 else f32
    npch = Jmax // 128  # positive-side partition chunks
    nw = K - nst  # negative-side free width

    nc = bacc.Bacc(
        "TRN2",
        target_bir_lowering=False,
        debug=False,
        enable_asserts=False,
        num_devices=N_CORES,
    )

    keys_d = nc.dram_tensor("keys_t", [BPC, D, K], kdt, kind="ExternalInput").ap()
    w1t_d = nc.dram_tensor("w1t", [D, H], kdt, kind="ExternalInput").ap()
    w2_d = nc.dram_tensor("w2c", [H, 1], kdt, kind="ExternalInput").ap()
    b1_d = nc.dram_tensor("b1c", [H, 1], f32, kind="ExternalInput").ap()
    ppr_d = nc.dram_tensor("ppr", [BPC, Jmax], f32, kind="ExternalInput").ap()
    negrow_d = nc.dram_tensor("negrow", [BPC, nw], f32, kind="ExternalInput").ap()
    out_d = nc.dram_tensor("acc_out", [128, BPC], f32, kind="ExternalOutput").ap()

    with tile.TileContext(nc) as tc:
        with (
            tc.tile_pool(name="const", bufs=1) as cpool,
            tc.tile_pool(name="keys", bufs=8) as kpool,
            tc.tile_pool(name="h", bufs=3) as hpool,
            tc.tile_pool(name="svec", bufs=2) as spool,
            tc.tile_pool(name="big", bufs=2) as bpool,
            tc.tile_pool(name="dscr", bufs=2, space="DRAM") as dpool,
            tc.tile_pool(name="hp", bufs=2, space="PSUM") as hp_pool,
            tc.tile_pool(name="sp", bufs=2, space="PSUM") as sp_pool,
        ):
            # ---- constants ----
            w1t_sb = cpool.tile([128, 8 * H], kdt, tag="w1t")
            for dc in range(8):
                nc.sync.dma_start(
                    w1t_sb[:, dc * H : (dc + 1) * H],
                    w1t_d[dc * 128 : (dc + 1) * 128, :],
                )
            w2_sb = cpool.tile([H, 1], kdt, tag="w2")
            nc.sync.dma_start(w2_sb[:], w2_d[:])
            b1_sb = cpool.tile([H, 1], f32, tag="b1")
            nc.sync.dma_start(b1_sb[:], b1_d[:])
            acc_sb = cpool.tile([128, BPC], f32, tag="acc")

            for b in range(BPC):
                # ---- phase A: MLP scores ----
                hp = hp_pool.tile([H, 1024], f32, tag="hp")
                for dc in range(8):
                    kt = kpool.tile([128, K], kdt, tag="keys")
                    nc.sync.dma_start(kt[:], keys_d[b, dc * 128 : (dc + 1) * 128, :])
                    w_sl = w1t_sb[:, dc * H : (dc + 1) * H]
                    nc.tensor.matmul(
                        hp[:, 0:512], lhsT=w_sl, rhs=kt[:, 0:512],
                        start=(dc == 0), stop=(dc == 7),
                    )
                    nc.tensor.matmul(
                        hp[:, 512:1024], lhsT=w_sl, rhs=kt[:, 512:1024],
                        start=(dc == 0), stop=(dc == 7),
                    )
                # relu(h + b1): PSUM -> SBUF on DVE (cast to kdt for matmul-2)
                h0 = hpool.tile([H, 512], kdt, tag="h")
                h1 = hpool.tile([H, 512], kdt, tag="h")
                nc.vector.tensor_scalar(
                    h0[:], hp[:, 0:512], b1_sb[:], 0.0,
                    op0=mybir.AluOpType.add, op1=mybir.AluOpType.max,
                )
                nc.vector.tensor_scalar(
                    h1[:], hp[:, 512:1024], b1_sb[:], 0.0,
                    op0=mybir.AluOpType.add, op1=mybir.AluOpType.max,
                )
                # s_row over the full row
                sr_ps = sp_pool.tile([1, 1024], f32, tag="sr")
                nc.tensor.matmul(
                    sr_ps[0:1, 0:512], lhsT=w2_sb[:], rhs=h0[:],
                    start=True, stop=True,
                )
                nc.tensor.matmul(
                    sr_ps[0:1, 512:1024], lhsT=w2_sb[:], rhs=h1[:],
                    start=True, stop=True,
                )
                # padded score rows -> one fused tile: [-(s_pos+pad), s_neg+pad]
                ppr_sb = spool.tile([1, Jmax], f32, tag="ppr")
                nc.sync.dma_start(ppr_sb[:], ppr_d[b : b + 1, :])
                ngr_sb = spool.tile([1, nw], f32, tag="ngr")
                nc.sync.dma_start(ngr_sb[:], negrow_d[b : b + 1, :])
                exin_sb = spool.tile([1, Jmax + nw], f32, tag="exin")
                nc.vector.scalar_tensor_tensor(
                    exin_sb[0:1, 0:Jmax], sr_ps[0:1, 0:Jmax], -1.0, ppr_sb[:],
                    op0=mybir.AluOpType.mult, op1=mybir.AluOpType.subtract,
                )
                nc.vector.tensor_add(
                    exin_sb[0:1, Jmax : Jmax + nw], sr_ps[0:1, nst:K], ngr_sb[:]
                )
                # single exp: e^{-(s_pos+pad)} | e^{s_neg+pad}
                eall_sb = spool.tile([1, Jmax + nw], f32, tag="eall")
                nc.scalar.activation(
                    eall_sb[:], exin_sb[:],
                    mybir.ActivationFunctionType.Exp, scale=1.0,
                )
                # broadcast e^{s_neg} across partitions (GPSIMD, off PE/ACT)
                ebc_sb = bpool.tile([128, nw], f32, tag="ebc")
                nc.gpsimd.partition_broadcast(
                    ebc_sb[:], eall_sb[0:1, Jmax : Jmax + nw]
                )
                # reshape e^{-s_pos} row -> [128, npch] via DRAM round-trip
                scr = dpool.tile([1, Jmax], f32, tag="scr")
                nc.sync.dma_start(scr[:], eall_sb[0:1, 0:Jmax])
                ecc_sb = spool.tile([128, npch], f32, tag="ecc")
                nc.sync.dma_start(
                    ecc_sb[:], scr[0:1, :].rearrange("a (c p) -> (a p) c", p=128)
                )
                # outer products on DVE + one Ln(x+1) with accumulate on ACT
                tall_sb = bpool.tile([128, npch * nw], f32, tag="tall")
                for c in range(npch):
                    nc.vector.tensor_scalar_mul(
                        tall_sb[:, c * nw : (c + 1) * nw], ebc_sb[:],
                        ecc_sb[:, c : c + 1],
                    )
                lout_sb = bpool.tile([128, npch * nw], f32, tag="lout")
                nc.scalar.activation(
                    lout_sb[:], tall_sb[:],
                    mybir.ActivationFunctionType.Ln,
                    bias=1.0, scale=1.0,
                    accum_out=acc_sb[:, b : b + 1],
                )

            nc.sync.dma_start(out_d[:], acc_sb[:])

    nc.compile()
    return nc


def kernel(keys, tgts, knn_tgts, mask, W1, b1, W2, b2, _profile=False):
    import ml_dtypes

    from concourse.bass_utils import run_bass_kernel_spmd

    keys = np.asarray(keys, dtype=np.float32)
    tgts = np.asarray(tgts)
    knn_tgts = np.asarray(knn_tgts)
    mask = np.asarray(mask).astype(bool)
    W1 = np.asarray(W1, dtype=np.float32)
    b1 = np.asarray(b1, dtype=np.float32)
    W2 = np.asarray(W2, dtype=np.float32)

    # ---- host-side label/permutation prep ----
    y = knn_tgts == tgts[:, None]
    pos = y & mask
    neg = (~y) & mask
    P = pos.sum(axis=1)
    N_ = neg.sum(axis=1)
    cnt = float((P.astype(np.int64) * N_.astype(np.int64)).sum())

    # stable order: positives, negatives, masked-out
    rank = np.where(pos, 0, np.where(neg, 1, 2)).astype(np.int8)
    order = np.argsort(rank, axis=1, kind="stable")  # [B, K]

    Pmax = int(P.max())
    Pmin = int(P.min())
    assert Pmax <= 512, f"positive count {Pmax} > 512 unsupported"
    Jmax = min(512, ((Pmax + 127) // 128) * 128)
    npch = Jmax // 128
    nst = min(Pmin, 512)  # negative free region start (s_row slice origin)
    nw = K - nst

    # permuted, transposed keys: [B, D, K]
    keys_perm = np.take_along_axis(keys, order[:, :, None], axis=1)  # [B, K, D]
    keys_t = np.ascontiguousarray(keys_perm.transpose(0, 2, 1))
    kdt = ml_dtypes.bfloat16 if USE_BF16 else np.float32
    keys_t = keys_t.astype(kdt)

    # pads in permuted coordinates
    kidx = np.arange(K)[None, :]
    pospad = np.where(kidx < P[:, None], 0.0, PAD).astype(np.float32)  # [B, K]
    negpad = np.where(
        (kidx >= P[:, None]) & (kidx < (P + N_)[:, None]), 0.0, -PAD
    ).astype(np.float32)
    ppr = np.ascontiguousarray(pospad[:, :Jmax])  # [B, Jmax]
    negrow = np.ascontiguousarray(negpad[:, nst:])  # [B, nw]

    w1t = np.ascontiguousarray(W1.T).astype(kdt)  # [D, H]
    w2c = np.ascontiguousarray(W2.reshape(1, H).T).astype(kdt)  # [H, 1]
    b1c = np.ascontiguousarray(b1.reshape(H, 1))

    key = (Jmax, nst, USE_BF16)
    if key not in _cache:
        _cache[key] = _build_program(Jmax, nst, USE_BF16)
    nc = _cache[key]

    in_maps = []
    for c in range(N_CORES):
        sl = slice(c * BPC, (c + 1) * BPC)
        in_maps.append(
            {
                "keys_t": keys_t[sl],
                "w1t": w1t,
                "w2c": w2c,
                "b1c": b1c,
                "ppr": ppr[sl],
                "negrow": negrow[sl],
            }
        )

    res = run_bass_kernel_spmd(
        nc, in_maps, list(range(N_CORES)), trace=bool(_profile)
    )
    total = 0.0
    for r in res.results:
        total += float(r["acc_out"].astype(np.float64).sum())
    if _profile:
        print(f"HW exec time: {res.exec_time_ns} ns")
        globals()["_last_results"] = res
    loss = np.float64(total) / np.float64(cnt)
    return np.array(loss, dtype=np.float32)
